# revision 3
# baseline (speedup 1.0000x reference)
"""CRF loss kernel for Trainium2 (Bass/Tile), 8-core data parallel.

Math (per batch row b):
  llh[b] = score[b] - logZ[b];  output = mean_b llh[b]

Denominator (logZ) via the *scaled linear-space* forward algorithm with a
4-segment rank-1 factorization that halves the serial depth vs the fwd/bwd
split (v7: 255 rounds -> v8: 127 rounds):

  Z = c^T N_511 ... N_1 p_0,  N_t = D_{e_t} A_s^T  (A_s = exp(T - C0)).
  Split t into 4 segments of 128.  Segment transfer operators contract at
  the Birkhoff rate tanh(diam/4) ~ 0.25/step (transitions are ~N(0,0.01)),
  so a 128-step product is rank-1 to ~1e-78: P_s ~= (P_s w)(z^T P_s)/(z^T P_s w).
  The two middle segments then need only one fwd and one bwd *vector*
  recursion each from constant probes (w = z = ones):
    Z ~= (g4.u3)(v3.u2)(v2.f1) / ((v3.w)(v2.w))
  f1 = P_1 p0 (fwd), u_s = P_s w (fwd), v_s^T = z^T P_s (bwd), g4^T = c^T P_4.
  Validated vs the exact forward pass in f32: max |dlogZ| ~ 2e-6.

  That is 6 lanes x 127 serial steps (vs 2 x 255).  Each lane is K=33 rows;
  three lanes stack on partitions (rows 0:33/33:66/66:99) into one tile, so
  6 lanes = 2 tiles of [99, 256cols], each advancing one step per round via
  a single matmul (block-diag 99x99 stationary) + one DVE multiply.  The two
  tiles ping-pong between TensorE and VectorE, hiding each other's latency.
  Bwd lanes use the pre-multiplied state form vt_t = e_t*(A_s vt_{t+1}) so
  ALL lanes share the same MM-then-multiply step (bwd stationary = A_s^T).

  Per-round critical path (measured on v7's cost structure): TT[256]
  (256cyc/0.96GHz + 125ns PSUM access) + sem hop + MM[256 cols]
  (173ns fixed SBUF latency + 256cyc/1.2GHz) + sem hop ~= 910ns, with DVE
  work 2x425 = 850ns/round just underneath -- latency-and-DVE co-bound.

Layout: emissions are uploaded PRE-TRANSPOSED k-major per tile-stream
([99, 128, 256] bf16: per slot s the three lanes' e_t rows stacked), so
every step tile lands in SBUF via one plain contiguous DMA (no on-device
transposes, no zero-pad band: lanes pack 0:99 with no dead rows).

The final 6 state vectors per batch column are DMA'd out raw (2x[99,256]
bf16); the stitch (5 dots through one extra A_s application) runs on host
in f64, with the 511 total A_s scale factors restored as +511*C0 in logZ.

Numerator: score[b] = sum_t em[b,t,tag[b,t]] + transitions along the tag
path + start/end terms -- pure index arithmetic (0.05% of FLOPs), computed
host-side in f64.  All O(B*T*K) math runs on device.

Sharding: pure data parallel over batch (2048 -> 8 cores x 256), small
tensors replicated; per-core partial outputs are combined on host.
"""

from contextlib import ExitStack

import numpy as np

import concourse.bass as bass
import concourse.bacc as bacc
import concourse.tile as tile
from concourse import mybir
from concourse.bass_utils import run_bass_kernel_spmd

import ml_dtypes

BF16 = ml_dtypes.bfloat16

F32 = mybir.dt.float32
BF = mybir.dt.bfloat16

# Problem constants
B_FULL, T_FULL, K = 2048, 512, 33
N_CORES = 8
BC = B_FULL // N_CORES  # 256 batch rows per core
NB = BC                 # batch columns per core in SBUF
NSEG = 4                # time segments (rank-1 seams between middle segs)
NS = T_FULL // NSEG     # serial steps per lane (= rounds)
NL = 3                  # lanes stacked per tile (3*K = 99 <= 128 partitions)
NR = NL * K             # used partition rows per tile
C0 = 3.9832             # per-step log-growth rescale (see module docstring)


def build_crf_module(W=16, prefetch=2):
    """Per-core Bass module: two 3-lane stacked scans, raw states out."""
    NBLK = NS // W
    assert NS % W == 0

    nc = bacc.Bacc()

    # ---- DRAM I/O (per-core shapes) ----
    # k-major stacked emission streams, host pre-arranged: [NR, NS, NB];
    # stream a slot s rows = e_s / e_{128+s} / e_{256+s} (fwd lanes),
    # stream b slot s rows = e_{255-s} / e_{383-s} / e_{511-s} (bwd lanes).
    em_d = [
        nc.dram_tensor(f"em{t}", [NR, NS, NB], BF, kind="ExternalInput")
        for t in range(2)
    ]
    # block-diag stationaries: expt0 = diag(A_s,A_s,A_s) (fwd lanes),
    # expt1 = diag(A_s^T,A_s^T,A_s^T) (bwd lanes); A_s = exp(T - C0), bf16.
    expt_d = [
        nc.dram_tensor(f"expt{t}", [NR, NR], BF, kind="ExternalInput")
        for t in range(2)
    ]
    # per-lane init vectors (f32): stream a rows = exp(start)/w'/w',
    # stream b rows = 1/1/exp(end)  (w' = A_s^T ones, probes w = z = ones)
    ese_d = [
        nc.dram_tensor(f"esev{t}", [NR], F32, kind="ExternalInput")
        for t in range(2)
    ]
    st_o = [
        nc.dram_tensor(f"st{t}_o", [NR, NB], BF, kind="ExternalOutput")
        for t in range(2)
    ]

    with tile.TileContext(nc) as tc, ExitStack() as ctx:
        singles = ctx.enter_context(tc.tile_pool(name="singles", bufs=1))
        q_pool = ctx.enter_context(tc.tile_pool(name="q", bufs=2, space="PSUM"))

        # ---------------- setup ----------------
        # All prologue-critical DMAs ride the GpSimd DMA queue, whose
        # completion semaphores land ~4us after the data (vs ~10us on the
        # sync queue); ordered so round 0's dependencies resolve earliest.
        expT = [
            singles.tile([NR, NR], BF, tag=f"expT{t}", name=f"expT{t}")
            for t in range(2)
        ]
        ese = [
            singles.tile([NR, 1], F32, tag=f"ese{t}", name=f"ese{t}")
            for t in range(2)
        ]
        for t in range(2):
            nc.gpsimd.dma_start(out=expT[t][:, :], in_=expt_d[t][:, :])
            nc.gpsimd.dma_start(out=ese[t][:, :], in_=ese_d[t][:])

        # persistent block tiles (per stream): raw rows 0:NR written by one
        # contiguous DMA per block; exp -> eT rows 0:NR.
        NRAW = 2
        NET = 3
        raw_bufs = [
            [
                singles.tile([NR, W * NB], BF, tag=f"raw{t}_{p}", name=f"raw{t}_{p}")
                for p in range(NRAW)
            ]
            for t in range(2)
        ]
        eT_bufs = [
            [
                singles.tile([NR, W * NB], BF, tag=f"eT{t}_{p}", name=f"eT{t}_{p}")
                for p in range(NET)
            ]
            for t in range(2)
        ]

        # persistent state ping-pong tiles per stream
        st = [
            [
                singles.tile([NR, NB], BF, tag=f"st{t}_{p}", name=f"st{t}_{p}")
                for p in range(2)
            ]
            for t in range(2)
        ]

        def load_block(t, j, exp_chunks=1):
            """DMA raw emissions block j of stream t (rows 0:NR in one shot),
            then exp into the eT buffer (optionally in chunks so the first
            slots become available sooner).  Block 0 rides the GpSimd queue,
            whose DMA-completion semaphores land ~4us after the data vs
            ~10us on the sync queue -- it gates the first scan round."""
            eng = nc.gpsimd if j == 0 else nc.sync
            raw = raw_bufs[t][j % NRAW]
            eng.dma_start(
                out=raw[0:NR, :],
                in_=bass.AP(
                    tensor=em_d[t], offset=j * W * NB, ap=[[NS * NB, NR], [1, W * NB]]
                ),
            )
            eT = eT_bufs[t][j % NET]
            step = W * NB // exp_chunks
            for i in range(exp_chunks):
                nc.scalar.activation(
                    eT[0:NR, i * step : (i + 1) * step],
                    raw[0:NR, i * step : (i + 1) * step],
                    mybir.ActivationFunctionType.Exp,
                )
            return eT

        # ---------------- pipeline ----------------
        # block 0 of both streams first (they gate round 0), exp'd in
        # chunks interleaved so both tiles' first slots land earliest.
        for t in range(2):
            eng = nc.gpsimd
            raw = raw_bufs[t][0]
            eng.dma_start(
                out=raw[0:NR, :],
                in_=bass.AP(tensor=em_d[t], offset=0, ap=[[NS * NB, NR], [1, W * NB]]),
            )
        CHUNKS0 = 4
        step0 = W * NB // CHUNKS0
        for i in range(CHUNKS0):
            for t in range(2):
                nc.scalar.activation(
                    eT_bufs[t][0][0:NR, i * step0 : (i + 1) * step0],
                    raw_bufs[t][0][0:NR, i * step0 : (i + 1) * step0],
                    mybir.ActivationFunctionType.Exp,
                )
        for j in range(1, min(prefetch, NBLK)):
            for t in range(2):
                load_block(t, j)

        eT_cur = [None, None]
        for s in range(NS):
            j, ls = divmod(s, W)
            if ls == 0:
                for t in range(2):
                    eT_cur[t] = eT_bufs[t][j % NET]
                if j + prefetch < NBLK:
                    for t in range(2):
                        load_block(t, j + prefetch)
            if s == 0:
                # init: state = e_slot0 * init_vec per-partition
                for t in range(2):
                    nc.vector.tensor_scalar(
                        out=st[t][0][0:NR, :],
                        in0=eT_cur[t][0:NR, 0:NB],
                        scalar1=ese[t][0:NR, :],
                        scalar2=None,
                        op0=mybir.AluOpType.mult,
                    )
                continue
            p = (s - 1) % 2
            for t in range(2):
                q = q_pool.tile([128, NB], F32, tag=f"q{t}", name=f"q{t}")
                nc.tensor.matmul(
                    out=q[0:NR, :],
                    lhsT=expT[t][0:NR, 0:NR],
                    rhs=st[t][p][0:NR, :],
                    start=True,
                    stop=True,
                )
                nc.vector.tensor_tensor(
                    st[t][1 - p][0:NR, :],
                    q[0:NR, :],
                    eT_cur[t][0:NR, ls * NB : (ls + 1) * NB],
                    mybir.AluOpType.mult,
                )

        # ---------------- tail: raw final states out ----------------
        pfin = (NS - 1) % 2
        for t in range(2):
            nc.gpsimd.dma_start(
                out=bass.AP(tensor=st_o[t], offset=0, ap=[[NB, NR], [1, NB]]),
                in_=st[t][pfin][0:NR, :],
            )

    nc.finalize()
    return nc


_CACHE = {}
LAST_RESULT = None


def _get_module():
    key = "v8"
    if key not in _CACHE:
        _CACHE[key] = build_crf_module()
    return _CACHE[key]


def _host_reference(emissions, tags, mask, start_transitions, end_transitions, transitions):
    """Pure-numpy fallback (unused for the all-ones mask the spec generates)."""
    em = emissions.astype(np.float64)
    mk = mask.astype(np.float64)
    B, T, K_ = em.shape
    b_idx = np.arange(B)
    tg = tags.astype(np.int64)
    score = start_transitions[tg[:, 0]].astype(np.float64) + em[b_idx, 0, tg[:, 0]]
    prev = tg[:, 0]
    for t in range(1, T):
        step = transitions[prev, tg[:, t]] + em[b_idx, t, tg[:, t]]
        score = score + step * mk[:, t]
        prev = np.where(mk[:, t] > 0, tg[:, t], prev)
    score = score + end_transitions[prev]

    def lse(x, axis):
        m = x.max(axis=axis, keepdims=True)
        return (m + np.log(np.exp(x - m).sum(axis=axis, keepdims=True))).squeeze(axis)

    alpha = start_transitions[None, :] + em[:, 0, :]
    for t in range(1, T):
        nxt = lse(alpha[:, :, None] + transitions[None, :, :].astype(np.float64) + em[:, t, None, :], axis=1)
        alpha = np.where(mk[:, t][:, None] > 0, nxt, alpha)
    logZ = lse(alpha + end_transitions[None, :], axis=1)
    return np.float32((score - logZ).mean())


def kernel(emissions, tags, mask, start_transitions, end_transitions, transitions):
    emissions = np.asarray(emissions, dtype=np.float32)
    tags_i = np.asarray(tags).astype(np.int64)
    mask_np = np.asarray(mask)
    start_np = np.asarray(start_transitions, dtype=np.float32)
    end_np = np.asarray(end_transitions, dtype=np.float32)
    trans_np = np.asarray(transitions, dtype=np.float32)

    if not mask_np.all():
        return _host_reference(
            emissions, tags_i, mask_np, start_np, end_np, trans_np
        )

    nc = _get_module()

    # host-precomputed O(K^2) constants
    A_s = np.exp(trans_np.astype(np.float64) - C0)  # [K, K], A_s[k,l]
    wp = A_s.T @ np.ones(K)                         # w' = A_s^T w (1 A_s count)
    expt0 = np.zeros((NR, NR), dtype=np.float64)
    expt1 = np.zeros((NR, NR), dtype=np.float64)
    for lane in range(NL):
        lo = lane * K
        expt0[lo : lo + K, lo : lo + K] = A_s      # fwd: q = A_s^T st
        expt1[lo : lo + K, lo : lo + K] = A_s.T    # bwd: q = A_s st
    expt0 = expt0.astype(BF16)
    expt1 = expt1.astype(BF16)
    esev0 = np.ones(NR, dtype=np.float64)
    esev0[0:K] = np.exp(start_np.astype(np.float64))
    esev0[K : 2 * K] = wp
    esev0[2 * K : 3 * K] = wp
    esev0 = esev0.astype(np.float32)
    esev1 = np.ones(NR, dtype=np.float64)
    esev1[2 * K : 3 * K] = np.exp(end_np.astype(np.float64))
    esev1 = esev1.astype(np.float32)

    SEG = T_FULL // NSEG  # 128
    in_maps = []
    for c in range(N_CORES):
        sl = slice(c * BC, (c + 1) * BC)
        em_bf = emissions[sl].astype(BF16)          # [BC, T, K]
        emT = em_bf.transpose(2, 1, 0)              # [K, T, BC] view
        ema = np.concatenate(
            [emT[:, 0:SEG], emT[:, SEG : 2 * SEG], emT[:, 2 * SEG : 3 * SEG]],
            axis=0,
        )  # [NR, NS, NB] fwd lanes: f1 / u2 / u3
        emb = np.concatenate(
            [
                emT[:, SEG : 2 * SEG][:, ::-1],
                emT[:, 2 * SEG : 3 * SEG][:, ::-1],
                emT[:, 3 * SEG : 4 * SEG][:, ::-1],
            ],
            axis=0,
        )  # [NR, NS, NB] bwd lanes: v~2 / v~3 / g~4
        in_maps.append(
            {
                "em0": np.ascontiguousarray(ema),
                "em1": np.ascontiguousarray(emb),
                "expt0": expt0,
                "expt1": expt1,
                "esev0": esev0,
                "esev1": esev1,
            }
        )

    import os

    trace = bool(int(os.environ.get("CRF_TRACE", "0")))
    res = run_bass_kernel_spmd(nc, in_maps, list(range(N_CORES)), trace=trace)
    global LAST_RESULT
    LAST_RESULT = res

    # host numerator: index arithmetic only (gathers along the tag path)
    b_idx = np.arange(B_FULL)[:, None]
    t_idx = np.arange(T_FULL)[None, :]
    em_path = emissions[b_idx, t_idx, tags_i].astype(np.float64)  # [B, T]
    score = (
        em_path.sum(axis=1)
        + start_np[tags_i[:, 0]].astype(np.float64)
        + end_np[tags_i[:, -1]].astype(np.float64)
        + trans_np[tags_i[:, :-1], tags_i[:, 1:]].astype(np.float64).sum(axis=1)
    )

    # host stitch (f64): Z = (g4.u3)(v3.u2)(v2.f1) / ((v3.w)(v2.w)),
    # v_s = A_s @ v~_s, g4 = A_s @ g~4.  Total A_s count: device 6*127,
    # host wp 2, stitch 3 -> net (767 num - 256 den) = 511 = T-1.
    llh_sum = 0.0
    for c in range(N_CORES):
        sl = slice(c * BC, (c + 1) * BC)
        sta = res.results[c]["st0_o"].reshape(NR, NB).astype(np.float64)
        stb = res.results[c]["st1_o"].reshape(NR, NB).astype(np.float64)
        F1, U2, U3 = sta[0:K], sta[K : 2 * K], sta[2 * K : 3 * K]
        Vt2, Vt3, Gt4 = stb[0:K], stb[K : 2 * K], stb[2 * K : 3 * K]
        V2 = A_s @ Vt2
        V3 = A_s @ Vt3
        G4 = A_s @ Gt4
        num = (G4 * U3).sum(0) * (V3 * U2).sum(0) * (V2 * F1).sum(0)
        den = V3.sum(0) * V2.sum(0)
        logZ = np.log(num) - np.log(den) + (T_FULL - 1) * C0
        llh_sum += (score[sl] - logZ).sum()
    return np.float32(llh_sum / B_FULL)


# revision 6
# speedup vs baseline: 1.0842x; 1.0842x over previous
"""CRF loss kernel for Trainium2 (Bass/Tile), 8-core data parallel.

Math (per batch row b):
  llh[b] = score[b] - logZ[b];  output = mean_b llh[b]

Denominator (logZ) via the *scaled linear-space* forward algorithm with a
4-segment rank-1 factorization that halves the serial depth vs the fwd/bwd
split (v7: 255 rounds -> v8: 127 rounds):

  Z = c^T N_511 ... N_1 p_0,  N_t = D_{e_t} A_s^T  (A_s = exp(T - C0)).
  Split t into 4 segments of 128.  Segment transfer operators contract at
  the Birkhoff rate tanh(diam/4) ~ 0.25/step (transitions are ~N(0,0.01)),
  so a 128-step product is rank-1 to ~1e-78: P_s ~= (P_s w)(z^T P_s)/(z^T P_s w).
  The two middle segments then need only one fwd and one bwd *vector*
  recursion each from constant probes (w = z = ones):
    Z ~= (g4.u3)(v3.u2)(v2.f1) / ((v3.w)(v2.w))
  f1 = P_1 p0 (fwd), u_s = P_s w (fwd), v_s^T = z^T P_s (bwd), g4^T = c^T P_4.
  Validated vs the exact forward pass in f32: max |dlogZ| ~ 2e-6.

  That is 6 lanes x 127 serial steps (vs 2 x 255).  Each lane is K=33 rows;
  three lanes stack on partitions (rows 0:33/33:66/66:99) into one tile, so
  6 lanes = 2 tiles of [99, 256cols], each advancing one step per round via
  a single matmul (block-diag 99x99 stationary) + one DVE multiply.  The two
  tiles ping-pong between TensorE and VectorE, hiding each other's latency.
  Bwd lanes use the pre-multiplied state form vt_t = e_t*(A_s vt_{t+1}) so
  ALL lanes share the same MM-then-multiply step (bwd stationary = A_s^T).

  Per-round critical path (measured on v7's cost structure): TT[256]
  (256cyc/0.96GHz + 125ns PSUM access) + sem hop + MM[256 cols]
  (173ns fixed SBUF latency + 256cyc/1.2GHz) + sem hop ~= 910ns, with DVE
  work 2x425 = 850ns/round just underneath -- latency-and-DVE co-bound.

Layout: emissions are uploaded PRE-TRANSPOSED k-major per tile-stream
([99, 128, 256] bf16: per slot s the three lanes' e_t rows stacked), so
every step tile lands in SBUF via one plain contiguous DMA (no on-device
transposes, no zero-pad band: lanes pack 0:99 with no dead rows).

The final 6 state vectors per batch column are DMA'd out raw (2x[99,256]
bf16); the stitch (5 dots through one extra A_s application) runs on host
in f64, with the 511 total A_s scale factors restored as +511*C0 in logZ.

Numerator: score[b] = sum_t em[b,t,tag[b,t]] + transitions along the tag
path + start/end terms -- pure index arithmetic (0.05% of FLOPs), computed
host-side in f64.  All O(B*T*K) math runs on device.

Sharding: pure data parallel over batch (2048 -> 8 cores x 256), small
tensors replicated; per-core partial outputs are combined on host.
"""

from contextlib import ExitStack

import numpy as np

import concourse.bass as bass
import concourse.bacc as bacc
import concourse.tile as tile
from concourse import mybir
from concourse.bass_utils import run_bass_kernel_spmd

import ml_dtypes

BF16 = ml_dtypes.bfloat16

F32 = mybir.dt.float32
BF = mybir.dt.bfloat16

# Problem constants
B_FULL, T_FULL, K = 2048, 512, 33
N_CORES = 8
BC = B_FULL // N_CORES  # 256 batch rows per core
NB = BC                 # batch columns per core in SBUF
NSEG = 4                # time segments (rank-1 seams between middle segs)
NS = T_FULL // NSEG     # serial steps per lane (= rounds)
NL = 3                  # lanes stacked per tile (3*K = 99 <= 128 partitions)
NR = NL * K             # used partition rows per tile
C0 = 3.9832             # per-step log-growth rescale (see module docstring)


def build_crf_module(W=8, prefetch=2):
    """Per-core Bass module: two 3-lane stacked scans, raw states out."""
    NBLK = NS // W
    assert NS % W == 0

    nc = bacc.Bacc()

    # ---- DRAM I/O (per-core shapes) ----
    # k-major stacked emission streams, host pre-arranged: [NR, NS, NB];
    # stream a slot s rows = e_s / e_{128+s} / e_{256+s} (fwd lanes),
    # stream b slot s rows = e_{255-s} / e_{383-s} / e_{511-s} (bwd lanes).
    em_d = [
        nc.dram_tensor(f"em{t}", [NR, NS, NB], BF, kind="ExternalInput")
        for t in range(2)
    ]
    # block-diag stationaries: expt0 = diag(A_s,A_s,A_s) (fwd lanes),
    # expt1 = diag(A_s^T,A_s^T,A_s^T) (bwd lanes); A_s = exp(T - C0), bf16.
    expt_d = [
        nc.dram_tensor(f"expt{t}", [NR, NR], BF, kind="ExternalInput")
        for t in range(2)
    ]
    # per-lane init vectors (f32): stream a rows = exp(start)/w'/w',
    # stream b rows = 1/1/exp(end)  (w' = A_s^T ones, probes w = z = ones)
    ese_d = [
        nc.dram_tensor(f"esev{t}", [NR], F32, kind="ExternalInput")
        for t in range(2)
    ]
    st_o = [
        nc.dram_tensor(f"st{t}_o", [NR, NB], BF, kind="ExternalOutput")
        for t in range(2)
    ]

    with tile.TileContext(nc) as tc, ExitStack() as ctx:
        singles = ctx.enter_context(tc.tile_pool(name="singles", bufs=1))
        q_pool = ctx.enter_context(tc.tile_pool(name="q", bufs=2, space="PSUM"))

        # ---------------- setup ----------------
        # All prologue-critical DMAs ride the GpSimd DMA queue, whose
        # completion semaphores land ~4us after the data (vs ~10us on the
        # sync queue); ordered so round 0's dependencies resolve earliest.
        expT = [
            singles.tile([NR, NR], BF, tag=f"expT{t}", name=f"expT{t}")
            for t in range(2)
        ]
        ese = [
            singles.tile([NR, 1], F32, tag=f"ese{t}", name=f"ese{t}")
            for t in range(2)
        ]

        # persistent block tiles (per stream): raw rows 0:NR written by one
        # contiguous DMA per block; exp -> eT rows 0:NR.
        NRAW = 2
        NET = 3
        raw_bufs = [
            [
                singles.tile([NR, W * NB], BF, tag=f"raw{t}_{p}", name=f"raw{t}_{p}")
                for p in range(NRAW)
            ]
            for t in range(2)
        ]
        eT_bufs = [
            [
                singles.tile([NR, W * NB], BF, tag=f"eT{t}_{p}", name=f"eT{t}_{p}")
                for p in range(NET)
            ]
            for t in range(2)
        ]

        # persistent state ping-pong tiles per stream
        st = [
            [
                singles.tile([NR, NB], BF, tag=f"st{t}_{p}", name=f"st{t}_{p}")
                for p in range(2)
            ]
            for t in range(2)
        ]

        def load_block(t, j, exp_chunks=1):
            """DMA raw emissions block j of stream t (rows 0:NR in one shot),
            then exp into the eT buffer (optionally in chunks so the first
            slots become available sooner).  Block 0 rides the GpSimd queue,
            whose DMA-completion semaphores land ~4us after the data vs
            ~10us on the sync queue -- it gates the first scan round."""
            eng = nc.gpsimd if j == 0 else nc.sync
            raw = raw_bufs[t][j % NRAW]
            eng.dma_start(
                out=raw[0:NR, :],
                in_=bass.AP(
                    tensor=em_d[t], offset=j * W * NB, ap=[[NS * NB, NR], [1, W * NB]]
                ),
            )
            eT = eT_bufs[t][j % NET]
            step = W * NB // exp_chunks
            for i in range(exp_chunks):
                nc.scalar.activation(
                    eT[0:NR, i * step : (i + 1) * step],
                    raw[0:NR, i * step : (i + 1) * step],
                    mybir.ActivationFunctionType.Exp,
                )
            return eT

        # ---------------- pipeline ----------------
        # block 0 of both streams FIRST on the gpsimd queue (its triggers
        # serialize at ~700ns each and its completion sems gate round 0);
        # the tiny constant DMAs follow (their data is needed ~1us later).
        for t in range(2):
            raw = raw_bufs[t][0]
            nc.gpsimd.dma_start(
                out=raw[0:NR, :],
                in_=bass.AP(tensor=em_d[t], offset=0, ap=[[NS * NB, NR], [1, W * NB]]),
            )
        for t in range(2):
            nc.gpsimd.dma_start(out=ese[t][:, :], in_=ese_d[t][:])
            nc.gpsimd.dma_start(out=expT[t][:, :], in_=expt_d[t][:, :])
        CHUNKS0 = 2
        step0 = W * NB // CHUNKS0
        for i in range(CHUNKS0):
            for t in range(2):
                nc.scalar.activation(
                    eT_bufs[t][0][0:NR, i * step0 : (i + 1) * step0],
                    raw_bufs[t][0][0:NR, i * step0 : (i + 1) * step0],
                    mybir.ActivationFunctionType.Exp,
                )
        for j in range(1, min(prefetch, NBLK)):
            for t in range(2):
                load_block(t, j)

        eT_cur = [None, None]
        for s in range(NS):
            j, ls = divmod(s, W)
            if ls == 0:
                for t in range(2):
                    eT_cur[t] = eT_bufs[t][j % NET]
                if j + prefetch < NBLK:
                    for t in range(2):
                        load_block(t, j + prefetch)
            if s == 0:
                # init: state = e_slot0 * init_vec per-partition
                for t in range(2):
                    nc.vector.tensor_scalar(
                        out=st[t][0][0:NR, :],
                        in0=eT_cur[t][0:NR, 0:NB],
                        scalar1=ese[t][0:NR, :],
                        scalar2=None,
                        op0=mybir.AluOpType.mult,
                    )
                continue
            p = (s - 1) % 2
            for t in range(2):
                q = q_pool.tile([128, NB], F32, tag=f"q{t}", name=f"q{t}")
                nc.tensor.matmul(
                    out=q[0:NR, :],
                    lhsT=expT[t][0:NR, 0:NR],
                    rhs=st[t][p][0:NR, :],
                    start=True,
                    stop=True,
                )
                nc.vector.tensor_tensor(
                    st[t][1 - p][0:NR, :],
                    q[0:NR, :],
                    eT_cur[t][0:NR, ls * NB : (ls + 1) * NB],
                    mybir.AluOpType.mult,
                )

        # ---------------- tail: raw final states out ----------------
        pfin = (NS - 1) % 2
        for t in range(2):
            nc.gpsimd.dma_start(
                out=bass.AP(tensor=st_o[t], offset=0, ap=[[NB, NR], [1, NB]]),
                in_=st[t][pfin][0:NR, :],
            )

    nc.finalize()
    return nc


_CACHE = {}
LAST_RESULT = None


def _get_module():
    key = "v8"
    if key not in _CACHE:
        _CACHE[key] = build_crf_module()
    return _CACHE[key]


def _host_reference(emissions, tags, mask, start_transitions, end_transitions, transitions):
    """Pure-numpy fallback (unused for the all-ones mask the spec generates)."""
    em = emissions.astype(np.float64)
    mk = mask.astype(np.float64)
    B, T, K_ = em.shape
    b_idx = np.arange(B)
    tg = tags.astype(np.int64)
    score = start_transitions[tg[:, 0]].astype(np.float64) + em[b_idx, 0, tg[:, 0]]
    prev = tg[:, 0]
    for t in range(1, T):
        step = transitions[prev, tg[:, t]] + em[b_idx, t, tg[:, t]]
        score = score + step * mk[:, t]
        prev = np.where(mk[:, t] > 0, tg[:, t], prev)
    score = score + end_transitions[prev]

    def lse(x, axis):
        m = x.max(axis=axis, keepdims=True)
        return (m + np.log(np.exp(x - m).sum(axis=axis, keepdims=True))).squeeze(axis)

    alpha = start_transitions[None, :] + em[:, 0, :]
    for t in range(1, T):
        nxt = lse(alpha[:, :, None] + transitions[None, :, :].astype(np.float64) + em[:, t, None, :], axis=1)
        alpha = np.where(mk[:, t][:, None] > 0, nxt, alpha)
    logZ = lse(alpha + end_transitions[None, :], axis=1)
    return np.float32((score - logZ).mean())


def kernel(emissions, tags, mask, start_transitions, end_transitions, transitions):
    emissions = np.asarray(emissions, dtype=np.float32)
    tags_i = np.asarray(tags).astype(np.int64)
    mask_np = np.asarray(mask)
    start_np = np.asarray(start_transitions, dtype=np.float32)
    end_np = np.asarray(end_transitions, dtype=np.float32)
    trans_np = np.asarray(transitions, dtype=np.float32)

    if not mask_np.all():
        return _host_reference(
            emissions, tags_i, mask_np, start_np, end_np, trans_np
        )

    nc = _get_module()

    # host-precomputed O(K^2) constants
    A_s = np.exp(trans_np.astype(np.float64) - C0)  # [K, K], A_s[k,l]
    wp = A_s.T @ np.ones(K)                         # w' = A_s^T w (1 A_s count)
    expt0 = np.zeros((NR, NR), dtype=np.float64)
    expt1 = np.zeros((NR, NR), dtype=np.float64)
    for lane in range(NL):
        lo = lane * K
        expt0[lo : lo + K, lo : lo + K] = A_s      # fwd: q = A_s^T st
        expt1[lo : lo + K, lo : lo + K] = A_s.T    # bwd: q = A_s st
    expt0 = expt0.astype(BF16)
    expt1 = expt1.astype(BF16)
    esev0 = np.ones(NR, dtype=np.float64)
    esev0[0:K] = np.exp(start_np.astype(np.float64))
    esev0[K : 2 * K] = wp
    esev0[2 * K : 3 * K] = wp
    esev0 = esev0.astype(np.float32)
    esev1 = np.ones(NR, dtype=np.float64)
    esev1[2 * K : 3 * K] = np.exp(end_np.astype(np.float64))
    esev1 = esev1.astype(np.float32)

    SEG = T_FULL // NSEG  # 128
    in_maps = []
    for c in range(N_CORES):
        sl = slice(c * BC, (c + 1) * BC)
        em_bf = emissions[sl].astype(BF16)          # [BC, T, K]
        emT = em_bf.transpose(2, 1, 0)              # [K, T, BC] view
        ema = np.concatenate(
            [emT[:, 0:SEG], emT[:, SEG : 2 * SEG], emT[:, 2 * SEG : 3 * SEG]],
            axis=0,
        )  # [NR, NS, NB] fwd lanes: f1 / u2 / u3
        emb = np.concatenate(
            [
                emT[:, SEG : 2 * SEG][:, ::-1],
                emT[:, 2 * SEG : 3 * SEG][:, ::-1],
                emT[:, 3 * SEG : 4 * SEG][:, ::-1],
            ],
            axis=0,
        )  # [NR, NS, NB] bwd lanes: v~2 / v~3 / g~4
        in_maps.append(
            {
                "em0": np.ascontiguousarray(ema),
                "em1": np.ascontiguousarray(emb),
                "expt0": expt0,
                "expt1": expt1,
                "esev0": esev0,
                "esev1": esev1,
            }
        )

    import os

    trace = bool(int(os.environ.get("CRF_TRACE", "0")))
    res = run_bass_kernel_spmd(nc, in_maps, list(range(N_CORES)), trace=trace)
    global LAST_RESULT
    LAST_RESULT = res

    # host numerator: index arithmetic only (gathers along the tag path)
    b_idx = np.arange(B_FULL)[:, None]
    t_idx = np.arange(T_FULL)[None, :]
    em_path = emissions[b_idx, t_idx, tags_i].astype(np.float64)  # [B, T]
    score = (
        em_path.sum(axis=1)
        + start_np[tags_i[:, 0]].astype(np.float64)
        + end_np[tags_i[:, -1]].astype(np.float64)
        + trans_np[tags_i[:, :-1], tags_i[:, 1:]].astype(np.float64).sum(axis=1)
    )

    # host stitch (f64): Z = (g4.u3)(v3.u2)(v2.f1) / ((v3.w)(v2.w)),
    # v_s = A_s @ v~_s, g4 = A_s @ g~4.  Total A_s count: device 6*127,
    # host wp 2, stitch 3 -> net (767 num - 256 den) = 511 = T-1.
    llh_sum = 0.0
    for c in range(N_CORES):
        sl = slice(c * BC, (c + 1) * BC)
        sta = res.results[c]["st0_o"].reshape(NR, NB).astype(np.float64)
        stb = res.results[c]["st1_o"].reshape(NR, NB).astype(np.float64)
        F1, U2, U3 = sta[0:K], sta[K : 2 * K], sta[2 * K : 3 * K]
        Vt2, Vt3, Gt4 = stb[0:K], stb[K : 2 * K], stb[2 * K : 3 * K]
        V2 = A_s @ Vt2
        V3 = A_s @ Vt3
        G4 = A_s @ Gt4
        num = (G4 * U3).sum(0) * (V3 * U2).sum(0) * (V2 * F1).sum(0)
        den = V3.sum(0) * V2.sum(0)
        logZ = np.log(num) - np.log(den) + (T_FULL - 1) * C0
        llh_sum += (score[sl] - logZ).sum()
    return np.float32(llh_sum / B_FULL)


# revision 20
# speedup vs baseline: 1.1016x; 1.0160x over previous
"""CRF loss kernel for Trainium2 (Bass/Tile), 8-core data parallel.

Math (per batch row b):
  llh[b] = score[b] - logZ[b];  output = mean_b llh[b]

Denominator (logZ) via the *scaled linear-space* forward algorithm with a
4-segment rank-1 factorization that halves the serial depth vs the fwd/bwd
split (v7: 255 rounds -> v8: 127 rounds):

  Z = c^T N_511 ... N_1 p_0,  N_t = D_{e_t} A_s^T  (A_s = exp(T - C0)).
  Split t into 4 segments of 128.  Segment transfer operators contract at
  the Birkhoff rate tanh(diam/4) ~ 0.25/step (transitions are ~N(0,0.01)),
  so a 128-step product is rank-1 to ~1e-78: P_s ~= (P_s w)(z^T P_s)/(z^T P_s w).
  The two middle segments then need only one fwd and one bwd *vector*
  recursion each from constant probes (w = z = ones):
    Z ~= (g4.u3)(v3.u2)(v2.f1) / ((v3.w)(v2.w))
  f1 = P_1 p0 (fwd), u_s = P_s w (fwd), v_s^T = z^T P_s (bwd), g4^T = c^T P_4.
  Validated vs the exact forward pass in f32: max |dlogZ| ~ 2e-6.

  That is 6 lanes x 127 serial steps (vs 2 x 255).  Each lane is K=33 rows;
  three lanes stack on partitions (rows 0:33/33:66/66:99) into one tile, so
  6 lanes = 2 tiles of [99, 256cols], each advancing one step per round via
  a single matmul (block-diag 99x99 stationary) + one DVE multiply.  The two
  tiles ping-pong between TensorE and VectorE, hiding each other's latency.
  Bwd lanes use the pre-multiplied state form vt_t = e_t*(A_s vt_{t+1}) so
  ALL lanes share the same MM-then-multiply step (bwd stationary = A_s^T).

  Per-round critical path (measured on v7's cost structure): TT[256]
  (256cyc/0.96GHz + 125ns PSUM access) + sem hop + MM[256 cols]
  (173ns fixed SBUF latency + 256cyc/1.2GHz) + sem hop ~= 910ns, with DVE
  work 2x425 = 850ns/round just underneath -- latency-and-DVE co-bound.

Layout: emissions are uploaded PRE-TRANSPOSED k-major per tile-stream
([99, 128, 256] bf16: per slot s the three lanes' e_t rows stacked), so
every step tile lands in SBUF via one plain contiguous DMA (no on-device
transposes, no zero-pad band: lanes pack 0:99 with no dead rows).

The final 6 state vectors per batch column are DMA'd out raw (2x[99,256]
bf16); the stitch (5 dots through one extra A_s application) runs on host
in f64, with the 511 total A_s scale factors restored as +511*C0 in logZ.

Numerator: score[b] = sum_t em[b,t,tag[b,t]] + transitions along the tag
path + start/end terms -- pure index arithmetic (0.05% of FLOPs), computed
host-side in f64.  All O(B*T*K) math runs on device.

Sharding: pure data parallel over batch (2048 -> 8 cores x 256), small
tensors replicated; per-core partial outputs are combined on host.
"""

from contextlib import ExitStack

import numpy as np

import concourse.bass as bass
import concourse.bacc as bacc
import concourse.tile as tile
from concourse import mybir
from concourse.bass_utils import run_bass_kernel_spmd

import ml_dtypes

BF16 = ml_dtypes.bfloat16

F32 = mybir.dt.float32
BF = mybir.dt.bfloat16

# Problem constants
B_FULL, T_FULL, K = 2048, 512, 33
N_CORES = 8
BC = B_FULL // N_CORES  # 256 batch rows per core
NB = BC                 # batch columns per core in SBUF
NSEG = 4                # time segments (rank-1 seams between middle segs)
NS = T_FULL // NSEG     # serial steps per lane (= rounds)
NL = 3                  # lanes stacked per tile (3*K = 99 <= 128 partitions)
NR = NL * K             # used partition rows per tile
C0 = 3.9832             # per-step log-growth rescale (see module docstring)


def build_crf_module(W=8, prefetch=2, pool_cols=0):
    """Per-core Bass module: two 3-lane stacked scans, raw states out.

    pool_cols: trailing columns of each per-round multiply offloaded to the
    Pool (GpSimd) engine.  Dead on TRN2: the BIR verifier rejects GPSIMD
    PSUM reads ("GPSIMD Instructions cannot access PSUM"), and ACT's
    activation scale/bias are per-partition scalars only -- the q*e multiply
    can only run on the DVE.  Kept for documentation."""
    NBLK = NS // W
    assert NS % W == 0

    nc = bacc.Bacc()

    # ---- DRAM I/O (per-core shapes) ----
    # k-major stacked emission streams, host pre-arranged: [NR, NS, NB];
    # stream a slot s rows = e_s / e_{128+s} / e_{256+s} (fwd lanes),
    # stream b slot s rows = e_{255-s} / e_{383-s} / e_{511-s} (bwd lanes).
    em_d = [
        nc.dram_tensor(f"em{t}", [NR, NS, NB], BF, kind="ExternalInput")
        for t in range(2)
    ]
    # O(K^2) constants packed into two DMAs (each completion sem costs
    # ~1.4us of gpsimd-queue serialization): bf16 stationaries, cols 0:NR =
    # expt0 = diag(A_s,A_s,A_s) (fwd lanes), NR:2NR = expt1 = diag(A_s^T x3)
    # (bwd lanes), A_s = exp(T - C0); f32 init vectors (tensor_scalar
    # requires f32), col 0 = exp(start)/w'/w', col 1 = 1/1/exp(end)
    # (w' = A_s^T ones, probes w = z = ones).
    consts_d = nc.dram_tensor("consts", [NR, 2 * NR], BF, kind="ExternalInput")
    ese_d = nc.dram_tensor("esev", [NR, 2], F32, kind="ExternalInput")
    st_o = [
        nc.dram_tensor(f"st{t}_o", [NR, NB], BF, kind="ExternalOutput")
        for t in range(2)
    ]

    with tile.TileContext(nc) as tc, ExitStack() as ctx:
        singles = ctx.enter_context(tc.tile_pool(name="singles", bufs=1))
        q_pool = ctx.enter_context(tc.tile_pool(name="q", bufs=2, space="PSUM"))

        # ---------------- setup ----------------
        # All prologue-critical DMAs ride the GpSimd DMA queue, whose
        # completion semaphores land ~4us after the data (vs ~10us on the
        # sync queue); ordered so round 0's dependencies resolve earliest.
        consts = singles.tile([NR, 2 * NR], BF, tag="consts", name="consts")
        expT = [consts[0:NR, t * NR : (t + 1) * NR] for t in range(2)]
        esev = singles.tile([NR, 2], F32, tag="esev", name="esev")
        ese = [esev[0:NR, t : t + 1] for t in range(2)]

        # persistent block tiles (per stream): raw rows 0:NR written by one
        # contiguous DMA per block; exp -> eT rows 0:NR.
        NRAW = 2
        NET = 3
        raw_bufs = [
            [
                singles.tile([NR, W * NB], BF, tag=f"raw{t}_{p}", name=f"raw{t}_{p}")
                for p in range(NRAW)
            ]
            for t in range(2)
        ]
        eT_bufs = [
            [
                singles.tile([NR, W * NB], BF, tag=f"eT{t}_{p}", name=f"eT{t}_{p}")
                for p in range(NET)
            ]
            for t in range(2)
        ]

        # persistent state ping-pong tiles per stream
        st = [
            [
                singles.tile([NR, NB], BF, tag=f"st{t}_{p}", name=f"st{t}_{p}")
                for p in range(2)
            ]
            for t in range(2)
        ]

        def load_block(t, j, exp_chunks=1):
            """DMA raw emissions block j of stream t (rows 0:NR in one shot),
            then exp into the eT buffer (optionally in chunks so the first
            slots become available sooner).  Block 0 rides the GpSimd queue,
            whose DMA-completion semaphores land ~4us after the data vs
            ~10us on the sync queue -- it gates the first scan round."""
            eng = nc.gpsimd if j <= 1 else nc.sync
            raw = raw_bufs[t][j % NRAW]
            eng.dma_start(
                out=raw[0:NR, :],
                in_=bass.AP(
                    tensor=em_d[t], offset=j * W * NB, ap=[[NS * NB, NR], [1, W * NB]]
                ),
            )
            eT = eT_bufs[t][j % NET]
            step = W * NB // exp_chunks
            for i in range(exp_chunks):
                nc.scalar.activation(
                    eT[0:NR, i * step : (i + 1) * step],
                    raw[0:NR, i * step : (i + 1) * step],
                    mybir.ActivationFunctionType.Exp,
                )
            return eT

        # ---------------- pipeline ----------------
        # gpsimd queue order = completion-sem order (~1.4us pipeline each):
        # consts (tiny, gates init+first MM), then block 0 of both streams
        # (gate round 0/1), then block 1 (keeps the sync-queue j>=2
        # prefetch transfers from contending with block 0's bandwidth).
        nc.gpsimd.dma_start(out=consts[:, :], in_=consts_d[:, :])
        nc.gpsimd.dma_start(out=esev[:, :], in_=ese_d[:, :])
        for t in range(2):
            raw = raw_bufs[t][0]
            nc.gpsimd.dma_start(
                out=raw[0:NR, :],
                in_=bass.AP(tensor=em_d[t], offset=0, ap=[[NS * NB, NR], [1, W * NB]]),
            )
        CHUNKS0 = 2
        step0 = W * NB // CHUNKS0
        for i in range(CHUNKS0):
            for t in range(2):
                nc.scalar.activation(
                    eT_bufs[t][0][0:NR, i * step0 : (i + 1) * step0],
                    raw_bufs[t][0][0:NR, i * step0 : (i + 1) * step0],
                    mybir.ActivationFunctionType.Exp,
                )
        for j in range(1, min(prefetch, NBLK)):
            for t in range(2):
                load_block(t, j)

        eT_cur = [None, None]
        for s in range(NS):
            j, ls = divmod(s, W)
            if ls == 0:
                for t in range(2):
                    eT_cur[t] = eT_bufs[t][j % NET]
                if j + prefetch < NBLK:
                    for t in range(2):
                        load_block(t, j + prefetch)
            if s == 0:
                # init: state = e_slot0 * init_vec per-partition
                for t in range(2):
                    nc.vector.tensor_scalar(
                        out=st[t][0][0:NR, :],
                        in0=eT_cur[t][0:NR, 0:NB],
                        scalar1=ese[t][0:NR, :],
                        scalar2=None,
                        op0=mybir.AluOpType.mult,
                    )
                continue
            p = (s - 1) % 2
            cd = NB - pool_cols
            for t in range(2):
                q = q_pool.tile([128, NB], F32, tag=f"q{t}", name=f"q{t}")
                nc.tensor.matmul(
                    out=q[0:NR, :],
                    lhsT=expT[t][0:NR, 0:NR],
                    rhs=st[t][p][0:NR, :],
                    start=True,
                    stop=True,
                )
                nc.vector.tensor_tensor(
                    st[t][1 - p][0:NR, 0:cd],
                    q[0:NR, 0:cd],
                    eT_cur[t][0:NR, ls * NB : ls * NB + cd],
                    mybir.AluOpType.mult,
                )
                if pool_cols:
                    nc.gpsimd.tensor_tensor(
                        st[t][1 - p][0:NR, cd:NB],
                        q[0:NR, cd:NB],
                        eT_cur[t][0:NR, ls * NB + cd : (ls + 1) * NB],
                        mybir.AluOpType.mult,
                    )

        # ---------------- tail: raw final states out ----------------
        pfin = (NS - 1) % 2
        for t in range(2):
            nc.gpsimd.dma_start(
                out=bass.AP(tensor=st_o[t], offset=0, ap=[[NB, NR], [1, NB]]),
                in_=st[t][pfin][0:NR, :],
            )

    nc.finalize()
    return nc


_CACHE = {}
LAST_RESULT = None


def _get_module():
    key = "v8"
    if key not in _CACHE:
        _CACHE[key] = build_crf_module()
    return _CACHE[key]


def _host_reference(emissions, tags, mask, start_transitions, end_transitions, transitions):
    """Pure-numpy fallback (unused for the all-ones mask the spec generates)."""
    em = emissions.astype(np.float64)
    mk = mask.astype(np.float64)
    B, T, K_ = em.shape
    b_idx = np.arange(B)
    tg = tags.astype(np.int64)
    score = start_transitions[tg[:, 0]].astype(np.float64) + em[b_idx, 0, tg[:, 0]]
    prev = tg[:, 0]
    for t in range(1, T):
        step = transitions[prev, tg[:, t]] + em[b_idx, t, tg[:, t]]
        score = score + step * mk[:, t]
        prev = np.where(mk[:, t] > 0, tg[:, t], prev)
    score = score + end_transitions[prev]

    def lse(x, axis):
        m = x.max(axis=axis, keepdims=True)
        return (m + np.log(np.exp(x - m).sum(axis=axis, keepdims=True))).squeeze(axis)

    alpha = start_transitions[None, :] + em[:, 0, :]
    for t in range(1, T):
        nxt = lse(alpha[:, :, None] + transitions[None, :, :].astype(np.float64) + em[:, t, None, :], axis=1)
        alpha = np.where(mk[:, t][:, None] > 0, nxt, alpha)
    logZ = lse(alpha + end_transitions[None, :], axis=1)
    return np.float32((score - logZ).mean())


def kernel(emissions, tags, mask, start_transitions, end_transitions, transitions):
    emissions = np.asarray(emissions, dtype=np.float32)
    tags_i = np.asarray(tags).astype(np.int64)
    mask_np = np.asarray(mask)
    start_np = np.asarray(start_transitions, dtype=np.float32)
    end_np = np.asarray(end_transitions, dtype=np.float32)
    trans_np = np.asarray(transitions, dtype=np.float32)

    if not mask_np.all():
        return _host_reference(
            emissions, tags_i, mask_np, start_np, end_np, trans_np
        )

    nc = _get_module()

    # host-precomputed O(K^2) constants, packed into one DMA payload
    A_s = np.exp(trans_np.astype(np.float64) - C0)  # [K, K], A_s[k,l]
    wp = A_s.T @ np.ones(K)                         # w' = A_s^T w (1 A_s count)
    consts = np.zeros((NR, 2 * NR), dtype=np.float64)
    for lane in range(NL):
        lo = lane * K
        consts[lo : lo + K, lo : lo + K] = A_s               # expt0: q = A_s^T st
        consts[lo : lo + K, NR + lo : NR + lo + K] = A_s.T   # expt1: q = A_s st
    consts = consts.astype(BF16)
    esev = np.ones((NR, 2), dtype=np.float64)
    esev[0:K, 0] = np.exp(start_np.astype(np.float64))
    esev[K : 2 * K, 0] = wp
    esev[2 * K : 3 * K, 0] = wp
    esev[2 * K : 3 * K, 1] = np.exp(end_np.astype(np.float64))
    esev = esev.astype(np.float32)

    SEG = T_FULL // NSEG  # 128
    in_maps = []
    for c in range(N_CORES):
        sl = slice(c * BC, (c + 1) * BC)
        em_bf = emissions[sl].astype(BF16)          # [BC, T, K]
        emT = em_bf.transpose(2, 1, 0)              # [K, T, BC] view
        ema = np.concatenate(
            [emT[:, 0:SEG], emT[:, SEG : 2 * SEG], emT[:, 2 * SEG : 3 * SEG]],
            axis=0,
        )  # [NR, NS, NB] fwd lanes: f1 / u2 / u3
        emb = np.concatenate(
            [
                emT[:, SEG : 2 * SEG][:, ::-1],
                emT[:, 2 * SEG : 3 * SEG][:, ::-1],
                emT[:, 3 * SEG : 4 * SEG][:, ::-1],
            ],
            axis=0,
        )  # [NR, NS, NB] bwd lanes: v~2 / v~3 / g~4
        in_maps.append(
            {
                "em0": np.ascontiguousarray(ema),
                "em1": np.ascontiguousarray(emb),
                "consts": consts,
                "esev": esev,
            }
        )

    import os

    trace = bool(int(os.environ.get("CRF_TRACE", "0")))
    res = run_bass_kernel_spmd(nc, in_maps, list(range(N_CORES)), trace=trace)
    global LAST_RESULT
    LAST_RESULT = res

    # host numerator: index arithmetic only (gathers along the tag path)
    b_idx = np.arange(B_FULL)[:, None]
    t_idx = np.arange(T_FULL)[None, :]
    em_path = emissions[b_idx, t_idx, tags_i].astype(np.float64)  # [B, T]
    score = (
        em_path.sum(axis=1)
        + start_np[tags_i[:, 0]].astype(np.float64)
        + end_np[tags_i[:, -1]].astype(np.float64)
        + trans_np[tags_i[:, :-1], tags_i[:, 1:]].astype(np.float64).sum(axis=1)
    )

    # host stitch (f64): Z = (g4.u3)(v3.u2)(v2.f1) / ((v3.w)(v2.w)),
    # v_s = A_s @ v~_s, g4 = A_s @ g~4.  Total A_s count: device 6*127,
    # host wp 2, stitch 3 -> net (767 num - 256 den) = 511 = T-1.
    llh_sum = 0.0
    for c in range(N_CORES):
        sl = slice(c * BC, (c + 1) * BC)
        sta = res.results[c]["st0_o"].reshape(NR, NB).astype(np.float64)
        stb = res.results[c]["st1_o"].reshape(NR, NB).astype(np.float64)
        F1, U2, U3 = sta[0:K], sta[K : 2 * K], sta[2 * K : 3 * K]
        Vt2, Vt3, Gt4 = stb[0:K], stb[K : 2 * K], stb[2 * K : 3 * K]
        V2 = A_s @ Vt2
        V3 = A_s @ Vt3
        G4 = A_s @ Gt4
        num = (G4 * U3).sum(0) * (V3 * U2).sum(0) * (V2 * F1).sum(0)
        den = V3.sum(0) * V2.sum(0)
        logZ = np.log(num) - np.log(den) + (T_FULL - 1) * C0
        llh_sum += (score[sl] - logZ).sum()
    return np.float32(llh_sum / B_FULL)
